# revision 12
# baseline (speedup 1.0000x reference)
"""Trainium2 Bass kernel for nn_ConvDS (2x2 pixel-unshuffle + 4x4 grouped 1x1 conv).

Reference math (scale=2, H=W=1024, no padding needed):
    xr[b,c,i,hs,ws] = x[b, c, 2*hs + i//2, 2*ws + i%2]        (i = 2*dy + dx)
    out[b, j*C + c, hs, ws] = sum_i W[j,i] * xr[b,c,i,hs,ws]

Sharding: pure data parallel over batch B=16 -> 2 images per core on 8 cores.

Per-core layout trick: view each [1024, 1024] image as [512, 2048] so one
SBUF partition holds an output row's two source rows contiguously:
    free dim = [r0 (1024 interleaved a,b)| r1 (1024 interleaved c,d)]
Stage 1 (VectorE): s1=a+b, d1=a-b, s2=c+d, d2=c-d   (stride-2 reads)
Stage 2 (VectorE): u0=s1+s2, u1=d1+d2, u2=s1-s2, u3=d1-d2
Stage 3 (ScalarE): out_j = rowscale_j * u_{combo_j}   (0.25 for Haar)
which equals W @ [a,b,c,d] for any W whose rows are scalar multiples of the
4-point Hadamard rows (the Haar weights are exactly that). A general-W
fallback path handles arbitrary conv_weights.
"""

import numpy as np

import concourse.mybir as mybir
import concourse.tile as tile
from concourse import bacc
from concourse.bass_utils import run_bass_kernel_spmd

N_CORES = 8
B, C, H, W = 16, 3, 1024, 1024
Hs, Ws = H // 2, W // 2  # 512, 512
BP = B // N_CORES  # batches per core
F32 = mybir.dt.float32

TILE_P = 128  # partitions (output rows hs) per tile
TILE_F = 2 * W  # free dim: two image rows per partition
N_TILES = Hs // TILE_P  # 4 row-tiles per image

# Hadamard sign rows in i = 2*dy + dx ordering (matches reference butterfly)
_HROWS = np.array(
    [
        [1.0, 1.0, 1.0, 1.0],
        [1.0, -1.0, 1.0, -1.0],
        [1.0, 1.0, -1.0, -1.0],
        [1.0, -1.0, -1.0, 1.0],
    ],
    dtype=np.float64,
)


def _match_hadamard(w):
    """If every row of w is (signed scalar) * a Hadamard sign row, return
    (combo_idx per row, signed scale per row); else None."""
    combos, scales = [], []
    for j in range(4):
        row = w[j].astype(np.float64)
        mag = np.abs(row)
        if mag[0] == 0 or not np.allclose(mag, mag[0], rtol=1e-6, atol=0):
            return None
        hit = None
        for k in range(4):
            if np.allclose(row, mag[0] * _HROWS[k], rtol=1e-6, atol=0):
                hit = (k, float(mag[0]))
                break
            if np.allclose(row, -mag[0] * _HROWS[k], rtol=1e-6, atol=0):
                hit = (k, float(-mag[0]))
                break
        if hit is None:
            return None
        combos.append(hit[0])
        scales.append(hit[1])
    return combos, scales


def _block_body(nc, sp, up, op, oview, X, c, t, had, w):
    """Emit compute + out-DMA for one [128, 2048] block X (one row-tile)."""
    # strided views of the four sub-pixel streams
    va = X[:, 0:W:2]
    vb = X[:, 1:W:2]
    vc = X[:, W : 2 * W : 2]
    vd = X[:, W + 1 : 2 * W : 2]

    O = op.tile([TILE_P, 4 * Ws], F32)
    if had is not None:
        combos, scales = had
        S = sp.tile([TILE_P, 4 * Ws], F32)
        nc.vector.tensor_add(S[:, 0 * Ws : 1 * Ws], va, vb)
        nc.vector.tensor_sub(S[:, 1 * Ws : 2 * Ws], va, vb)
        nc.vector.tensor_add(S[:, 2 * Ws : 3 * Ws], vc, vd)
        nc.vector.tensor_sub(S[:, 3 * Ws : 4 * Ws], vc, vd)
        s1 = S[:, 0 * Ws : 1 * Ws]
        d1 = S[:, 1 * Ws : 2 * Ws]
        s2 = S[:, 2 * Ws : 3 * Ws]
        d2 = S[:, 3 * Ws : 4 * Ws]
        U = up.tile([TILE_P, 4 * Ws], F32)
        for k in sorted(set(combos)):
            dst = U[:, k * Ws : (k + 1) * Ws]
            if k == 0:
                nc.vector.tensor_add(dst, s1, s2)
            elif k == 1:
                nc.vector.tensor_add(dst, d1, d2)
            elif k == 2:
                nc.vector.tensor_sub(dst, s1, s2)
            else:
                nc.vector.tensor_sub(dst, d1, d2)
        for j in range(4):
            k = combos[j]
            nc.scalar.mul(
                O[:, j * Ws : (j + 1) * Ws],
                U[:, k * Ws : (k + 1) * Ws],
                scales[j],
            )
    else:
        # General 4x4 weights fallback.
        T = sp.tile([TILE_P, 4 * Ws], F32)
        U = up.tile([TILE_P, 2 * Ws], F32)
        vs = (va, vb, vc, vd)
        for j in range(4):
            for i in range(4):
                nc.vector.tensor_scalar_mul(
                    T[:, i * Ws : (i + 1) * Ws], vs[i], float(w[j, i])
                )
            nc.vector.tensor_add(U[:, 0:Ws], T[:, 0:Ws], T[:, Ws : 2 * Ws])
            nc.vector.tensor_add(
                U[:, Ws : 2 * Ws], T[:, 2 * Ws : 3 * Ws], T[:, 3 * Ws : 4 * Ws]
            )
            nc.vector.tensor_add(
                O[:, j * Ws : (j + 1) * Ws], U[:, 0:Ws], U[:, Ws : 2 * Ws]
            )

    # one DMA out: SBUF [p, (j w)] -> DRAM [h, j, w]
    nc.scalar.dma_start(
        oview[c, t * TILE_P : (t + 1) * TILE_P],
        O[:].rearrange("p (j w) -> p j w", j=4),
    )


def _build(w, bufs=3, blocks_per_dma=1):
    """Build the per-core Bass program. w: host numpy [4,4] weights.

    blocks_per_dma: how many 128-row blocks one input DMA covers (the X tile
    is [128, blocks_per_dma*2048]; compute still runs per 2048-wide block).
    """
    nc = bacc.Bacc(None)
    # input viewed as [BP, C, Hs, 2*W]: partition rows are output rows hs,
    # each holding its two source image rows contiguously.
    xd = nc.dram_tensor("x", [BP, C, Hs, TILE_F], F32, kind="ExternalInput")
    od = nc.dram_tensor("out", [BP, 4 * C, Hs, Ws], F32, kind="ExternalOutput")

    had = _match_hadamard(w)
    bpd = blocks_per_dma
    assert N_TILES % bpd == 0

    with tile.TileContext(nc) as tc:
        with (
            tc.tile_pool(name="xp", bufs=bufs) as xp,
            tc.tile_pool(name="sp", bufs=bufs) as sp,
            tc.tile_pool(name="up", bufs=bufs) as up,
            tc.tile_pool(name="op", bufs=bufs) as op,
        ):
            for b in range(BP):
                for c in range(C):
                    # DRAM output view: [c, h, j, w] with channel = j*C + c
                    oview = od[b].rearrange("(j c2) h w -> c2 h j w", j=4)
                    for tg in range(N_TILES // bpd):
                        Xg = xp.tile([TILE_P, bpd * TILE_F], F32)
                        src = xd[
                            b, c, tg * bpd * TILE_P : (tg + 1) * bpd * TILE_P, :
                        ].rearrange("(k p) f -> p k f", k=bpd)
                        nc.sync.dma_start(
                            Xg[:].rearrange("p (k f) -> p k f", k=bpd), src
                        )
                        for k in range(bpd):
                            t = tg * bpd + k
                            X = Xg[:, k * TILE_F : (k + 1) * TILE_F]
                            _block_body(nc, sp, up, op, oview, X, c, t, had, w)
    nc.compile()
    return nc


_CACHE = {}


def _get_program(w):
    key = w.tobytes()
    if key not in _CACHE:
        _CACHE[key] = _build(w)
    return _CACHE[key]


def _run(x, conv_weights, **spmd_kwargs):
    x = np.ascontiguousarray(np.asarray(x, dtype=np.float32))
    w = np.asarray(conv_weights, dtype=np.float32)
    assert x.shape == (B, C, H, W), x.shape
    nc = _get_program(w)
    in_maps = [
        {"x": x[k * BP : (k + 1) * BP].reshape(BP, C, Hs, TILE_F)}
        for k in range(N_CORES)
    ]
    res = run_bass_kernel_spmd(nc, in_maps, list(range(N_CORES)), **spmd_kwargs)
    out = np.concatenate([res.results[k]["out"] for k in range(N_CORES)], axis=0)
    return out.astype(np.float32, copy=False), res


def kernel(x, conv_weights):
    out, _ = _run(x, conv_weights)
    return out


def kernel_timed(x, conv_weights, **spmd_kwargs):
    """Run with NTFF profiling; returns (out, BassKernelResults)."""
    return _run(x, conv_weights, trace=True, **spmd_kwargs)


# revision 13
# speedup vs baseline: 1.0120x; 1.0120x over previous
"""Trainium2 Bass kernel for nn_ConvDS (2x2 pixel-unshuffle + 4x4 grouped 1x1 conv).

Reference math (scale=2, H=W=1024, no padding needed):
    xr[b,c,i,hs,ws] = x[b, c, 2*hs + i//2, 2*ws + i%2]        (i = 2*dy + dx)
    out[b, j*C + c, hs, ws] = sum_i W[j,i] * xr[b,c,i,hs,ws]

Sharding: pure data parallel over batch B=16 -> 2 images per core on 8 cores.

Per-core layout trick: view each [1024, 1024] image as [512, 2048] so one
SBUF partition holds an output row's two source rows contiguously:
    free dim = [r0 (1024 interleaved a,b)| r1 (1024 interleaved c,d)]
Stage 1 (VectorE): s1=a+b, d1=a-b, s2=c+d, d2=c-d   (stride-2 reads)
Stage 2 (VectorE): u0=s1+s2, u1=d1+d2, u2=s1-s2, u3=d1-d2
Stage 3 (ScalarE): out_j = rowscale_j * u_{combo_j}   (0.25 for Haar)
which equals W @ [a,b,c,d] for any W whose rows are scalar multiples of the
4-point Hadamard rows (the Haar weights are exactly that). A general-W
fallback path handles arbitrary conv_weights.
"""

import numpy as np

import concourse.mybir as mybir
import concourse.tile as tile
from concourse import bacc
from concourse.bass_utils import run_bass_kernel_spmd

N_CORES = 8
B, C, H, W = 16, 3, 1024, 1024
Hs, Ws = H // 2, W // 2  # 512, 512
BP = B // N_CORES  # batches per core
F32 = mybir.dt.float32

TILE_P = 128  # partitions (output rows hs) per tile
TILE_F = 2 * W  # free dim: two image rows per partition
N_TILES = Hs // TILE_P  # 4 row-tiles per image

# Hadamard sign rows in i = 2*dy + dx ordering (matches reference butterfly)
_HROWS = np.array(
    [
        [1.0, 1.0, 1.0, 1.0],
        [1.0, -1.0, 1.0, -1.0],
        [1.0, 1.0, -1.0, -1.0],
        [1.0, -1.0, -1.0, 1.0],
    ],
    dtype=np.float64,
)


def _match_hadamard(w):
    """If every row of w is (signed scalar) * a Hadamard sign row, return
    (combo_idx per row, signed scale per row); else None."""
    combos, scales = [], []
    for j in range(4):
        row = w[j].astype(np.float64)
        mag = np.abs(row)
        if mag[0] == 0 or not np.allclose(mag, mag[0], rtol=1e-6, atol=0):
            return None
        hit = None
        for k in range(4):
            if np.allclose(row, mag[0] * _HROWS[k], rtol=1e-6, atol=0):
                hit = (k, float(mag[0]))
                break
            if np.allclose(row, -mag[0] * _HROWS[k], rtol=1e-6, atol=0):
                hit = (k, float(-mag[0]))
                break
        if hit is None:
            return None
        combos.append(hit[0])
        scales.append(hit[1])
    return combos, scales


def _block_body(nc, sp, up, op, oview, X, c, t, had, w):
    """Emit compute + out-DMA for one [128, 2048] block X (one row-tile)."""
    # strided views of the four sub-pixel streams
    va = X[:, 0:W:2]
    vb = X[:, 1:W:2]
    vc = X[:, W : 2 * W : 2]
    vd = X[:, W + 1 : 2 * W : 2]

    O = op.tile([TILE_P, 4 * Ws], F32)
    if had is not None:
        combos, scales = had
        # Fused butterfly: 4 wide TT ops instead of 8 narrow ones.
        # ac = [a | c], bd = [b | d]  (stride-2 over the whole 2048)
        ac = X[:, 0 : TILE_F : 2]
        bd = X[:, 1 : TILE_F : 2]
        S = sp.tile([TILE_P, 4 * Ws], F32)
        nc.vector.tensor_add(S[:, 0 : 2 * Ws], ac, bd)  # [s1 | s2]
        nc.vector.tensor_sub(S[:, 2 * Ws : 4 * Ws], ac, bd)  # [d1 | d2]
        # stage 2: in0 = [s1, d1], in1 = [s2, d2]  (3D APs over S)
        S4 = S[:].rearrange("p (g h w) -> p g h w", g=2, h=2)
        in0 = S4[:, :, 0]  # [128, 2, 512]
        in1 = S4[:, :, 1]
        U = up.tile([TILE_P, 4 * Ws], F32)
        U4 = U[:].rearrange("p (g h w) -> p g h w", g=2, h=2)
        nc.vector.tensor_add(U4[:, 0], in0, in1)  # [u0 | u1]
        nc.vector.tensor_sub(U4[:, 1], in0, in1)  # [u2 | u3]
        if combos == [0, 1, 2, 3] and len(set(scales)) == 1:
            nc.scalar.mul(O[:], U[:], scales[0])
        else:
            for j in range(4):
                k = combos[j]
                nc.scalar.mul(
                    O[:, j * Ws : (j + 1) * Ws],
                    U[:, k * Ws : (k + 1) * Ws],
                    scales[j],
                )
    else:
        # General 4x4 weights fallback.
        T = sp.tile([TILE_P, 4 * Ws], F32)
        U = up.tile([TILE_P, 2 * Ws], F32)
        vs = (va, vb, vc, vd)
        for j in range(4):
            for i in range(4):
                nc.vector.tensor_scalar_mul(
                    T[:, i * Ws : (i + 1) * Ws], vs[i], float(w[j, i])
                )
            nc.vector.tensor_add(U[:, 0:Ws], T[:, 0:Ws], T[:, Ws : 2 * Ws])
            nc.vector.tensor_add(
                U[:, Ws : 2 * Ws], T[:, 2 * Ws : 3 * Ws], T[:, 3 * Ws : 4 * Ws]
            )
            nc.vector.tensor_add(
                O[:, j * Ws : (j + 1) * Ws], U[:, 0:Ws], U[:, Ws : 2 * Ws]
            )

    # one DMA out: SBUF [p, (j w)] -> DRAM [h, j, w]
    nc.scalar.dma_start(
        oview[c, t * TILE_P : (t + 1) * TILE_P],
        O[:].rearrange("p (j w) -> p j w", j=4),
    )


def _build(w, bufs=3, blocks_per_dma=1):
    """Build the per-core Bass program. w: host numpy [4,4] weights.

    blocks_per_dma: how many 128-row blocks one input DMA covers (the X tile
    is [128, blocks_per_dma*2048]; compute still runs per 2048-wide block).
    """
    nc = bacc.Bacc(None)
    # input viewed as [BP, C, Hs, 2*W]: partition rows are output rows hs,
    # each holding its two source image rows contiguously.
    xd = nc.dram_tensor("x", [BP, C, Hs, TILE_F], F32, kind="ExternalInput")
    od = nc.dram_tensor("out", [BP, 4 * C, Hs, Ws], F32, kind="ExternalOutput")

    had = _match_hadamard(w)
    bpd = blocks_per_dma
    assert N_TILES % bpd == 0

    with tile.TileContext(nc) as tc:
        with (
            tc.tile_pool(name="xp", bufs=bufs) as xp,
            tc.tile_pool(name="sp", bufs=bufs) as sp,
            tc.tile_pool(name="up", bufs=bufs) as up,
            tc.tile_pool(name="op", bufs=bufs) as op,
        ):
            for b in range(BP):
                for c in range(C):
                    # DRAM output view: [c, h, j, w] with channel = j*C + c
                    oview = od[b].rearrange("(j c2) h w -> c2 h j w", j=4)
                    for tg in range(N_TILES // bpd):
                        Xg = xp.tile([TILE_P, bpd * TILE_F], F32)
                        src = xd[
                            b, c, tg * bpd * TILE_P : (tg + 1) * bpd * TILE_P, :
                        ].rearrange("(k p) f -> p k f", k=bpd)
                        nc.sync.dma_start(
                            Xg[:].rearrange("p (k f) -> p k f", k=bpd), src
                        )
                        for k in range(bpd):
                            t = tg * bpd + k
                            X = Xg[:, k * TILE_F : (k + 1) * TILE_F]
                            _block_body(nc, sp, up, op, oview, X, c, t, had, w)
    nc.compile()
    return nc


_CACHE = {}


def _get_program(w):
    key = w.tobytes()
    if key not in _CACHE:
        _CACHE[key] = _build(w)
    return _CACHE[key]


def _run(x, conv_weights, **spmd_kwargs):
    x = np.ascontiguousarray(np.asarray(x, dtype=np.float32))
    w = np.asarray(conv_weights, dtype=np.float32)
    assert x.shape == (B, C, H, W), x.shape
    nc = _get_program(w)
    in_maps = [
        {"x": x[k * BP : (k + 1) * BP].reshape(BP, C, Hs, TILE_F)}
        for k in range(N_CORES)
    ]
    res = run_bass_kernel_spmd(nc, in_maps, list(range(N_CORES)), **spmd_kwargs)
    out = np.concatenate([res.results[k]["out"] for k in range(N_CORES)], axis=0)
    return out.astype(np.float32, copy=False), res


def kernel(x, conv_weights):
    out, _ = _run(x, conv_weights)
    return out


def kernel_timed(x, conv_weights, **spmd_kwargs):
    """Run with NTFF profiling; returns (out, BassKernelResults)."""
    return _run(x, conv_weights, trace=True, **spmd_kwargs)


# revision 18
# speedup vs baseline: 1.0403x; 1.0280x over previous
"""Trainium2 Bass kernel for nn_ConvDS (2x2 pixel-unshuffle + 4x4 grouped 1x1 conv).

Reference math (scale=2, H=W=1024, no padding needed):
    xr[b,c,i,hs,ws] = x[b, c, 2*hs + i//2, 2*ws + i%2]        (i = 2*dy + dx)
    out[b, j*C + c, hs, ws] = sum_i W[j,i] * xr[b,c,i,hs,ws]

Sharding: pure data parallel over batch B=16 -> 2 images per core on 8 cores.

Per-core layout trick: view each [1024, 1024] image as [512, 2048] so one
SBUF partition holds an output row's two source rows contiguously:
    free dim = [r0 (1024 interleaved a,b) | r1 (1024 interleaved c,d)]
VectorE Haar butterfly over stride-2 views (2 ops/element, the minimum for
an exact 4-point Hadamard transform), ScalarE applies the per-row scales
(0.25 for Haar), HWDGE DMAs in/out. This handles any conv_weights whose
rows are scalar multiples of Hadamard rows; a general-W fallback covers
arbitrary weights.
"""

import numpy as np

import concourse.mybir as mybir
import concourse.tile as tile
from concourse import bacc
from concourse.bass_utils import run_bass_kernel_spmd

N_CORES = 8
B, C, H, W = 16, 3, 1024, 1024
Hs, Ws = H // 2, W // 2  # 512, 512
BP = B // N_CORES  # batches per core
F32 = mybir.dt.float32

TILE_P = 128  # partitions (output rows hs) per block
BLK_F = 2 * W  # free dim per block: two image rows per partition
N_BLOCKS = Hs // TILE_P  # 4 row-blocks per image

# Hadamard sign rows in i = 2*dy + dx ordering (matches reference butterfly)
_HROWS = np.array(
    [
        [1.0, 1.0, 1.0, 1.0],
        [1.0, -1.0, 1.0, -1.0],
        [1.0, 1.0, -1.0, -1.0],
        [1.0, -1.0, -1.0, 1.0],
    ],
    dtype=np.float64,
)


def _match_hadamard(w):
    """If every row of w is (signed scalar) * a Hadamard sign row, return
    (combo_idx per row, signed scale per row); else None."""
    combos, scales = [], []
    for j in range(4):
        row = w[j].astype(np.float64)
        mag = np.abs(row)
        if mag[0] == 0 or not np.allclose(mag, mag[0], rtol=1e-6, atol=0):
            return None
        hit = None
        for k in range(4):
            if np.allclose(row, mag[0] * _HROWS[k], rtol=1e-6, atol=0):
                hit = (k, float(mag[0]))
                break
            if np.allclose(row, -mag[0] * _HROWS[k], rtol=1e-6, atol=0):
                hit = (k, float(-mag[0]))
                break
        if hit is None:
            return None
        combos.append(hit[0])
        scales.append(hit[1])
    return combos, scales


def _general_body(nc, sp, up, op, oview, X, c, t, w):
    """General 4x4 weights fallback for one [128, 2048] block."""
    va = X[:, 0:W:2]
    vb = X[:, 1:W:2]
    vc = X[:, W : 2 * W : 2]
    vd = X[:, W + 1 : 2 * W : 2]
    O = op.tile([TILE_P, 4 * Ws], F32)
    T = sp.tile([TILE_P, 4 * Ws], F32)
    U = up.tile([TILE_P, 2 * Ws], F32)
    vs = (va, vb, vc, vd)
    for j in range(4):
        for i in range(4):
            nc.vector.tensor_scalar_mul(
                T[:, i * Ws : (i + 1) * Ws], vs[i], float(w[j, i])
            )
        nc.vector.tensor_add(U[:, 0:Ws], T[:, 0:Ws], T[:, Ws : 2 * Ws])
        nc.vector.tensor_add(
            U[:, Ws : 2 * Ws], T[:, 2 * Ws : 3 * Ws], T[:, 3 * Ws : 4 * Ws]
        )
        nc.vector.tensor_add(
            O[:, j * Ws : (j + 1) * Ws], U[:, 0:Ws], U[:, Ws : 2 * Ws]
        )
    nc.scalar.dma_start(
        oview[c, t * TILE_P : (t + 1) * TILE_P],
        O[:].rearrange("p (j w) -> p j w", j=4),
    )


def _build(w, bufs=6, fuse=1, xbufs=None):
    """Build the per-core Bass program. w: host numpy [4,4] weights.

    fuse: how many 128-row blocks one DMA / one DVE op covers.
    xbufs: input-tile buffer count (prefetch depth); defaults to bufs.
    """
    nc = bacc.Bacc(None)
    # input viewed as [BP, C, Hs, 2*W]: partition rows are output rows hs,
    # each holding its two source image rows contiguously.
    xd = nc.dram_tensor("x", [BP, C, Hs, BLK_F], F32, kind="ExternalInput")
    od = nc.dram_tensor("out", [BP, 4 * C, Hs, Ws], F32, kind="ExternalOutput")

    had = _match_hadamard(w)
    f = fuse
    assert N_BLOCKS % f == 0

    with tile.TileContext(nc) as tc:
        with (
            tc.tile_pool(name="xp", bufs=xbufs or bufs) as xp,
            tc.tile_pool(name="sp", bufs=bufs) as sp,
            tc.tile_pool(name="up", bufs=bufs) as up,
            tc.tile_pool(name="op", bufs=bufs) as op,
        ):
            for b in range(BP):
                for c in range(C):
                    # DRAM output view: [c, h, j, w] with channel = j*C + c
                    oview = od[b].rearrange("(j c2) h w -> c2 h j w", j=4)
                    for tg in range(N_BLOCKS // f):
                        X = xp.tile([TILE_P, f * BLK_F], F32)
                        src = xd[
                            b, c, tg * f * TILE_P : (tg + 1) * f * TILE_P, :
                        ].rearrange("(k p) g -> p k g", k=f)
                        nc.sync.dma_start(
                            X[:].rearrange("p (k g) -> p k g", k=f), src
                        )
                        if had is None:
                            for k in range(f):
                                _general_body(
                                    nc, sp, up, op, oview,
                                    X[:, k * BLK_F : (k + 1) * BLK_F],
                                    c, tg * f + k, w,
                                )
                            continue

                        combos, scales = had
                        # Fused Haar butterfly over f blocks at once.
                        # evens = [a_0 c_0 a_1 c_1 ...], odds = [b_0 d_0 ...]
                        ac = X[:, 0 : f * BLK_F : 2]
                        bd = X[:, 1 : f * BLK_F : 2]
                        S = sp.tile([TILE_P, f * 4 * Ws], F32)
                        half = f * 2 * Ws
                        nc.vector.tensor_add(S[:, 0:half], ac, bd)
                        nc.vector.tensor_sub(S[:, half : 2 * half], ac, bd)
                        # S layout: (g: s/d half, k: block, h: 1/2, w)
                        Sv = S[:].rearrange(
                            "p (g k h w) -> p k g h w", g=2, k=f, h=2
                        )
                        in0 = Sv[:, :, :, 0]  # [p, k, g, w]: s1_k, d1_k
                        in1 = Sv[:, :, :, 1]  # s2_k, d2_k
                        U = up.tile([TILE_P, f * 4 * Ws], F32)
                        Uv = U[:].rearrange("p (k j w) -> p k j w", k=f, j=4)
                        nc.vector.tensor_add(Uv[:, :, 0:2], in0, in1)
                        nc.vector.tensor_sub(Uv[:, :, 2:4], in0, in1)
                        O = op.tile([TILE_P, f * 4 * Ws], F32)
                        if combos == [0, 1, 2, 3] and len(set(scales)) == 1:
                            nc.scalar.mul(O[:], U[:], scales[0])
                        else:
                            for j in range(4):
                                k = combos[j]
                                for blk in range(f):
                                    jo = (blk * 4 + j) * Ws
                                    ko = (blk * 4 + k) * Ws
                                    nc.scalar.mul(
                                        O[:, jo : jo + Ws],
                                        U[:, ko : ko + Ws],
                                        scales[j],
                                    )
                        # DMA out per block: SBUF [p, (j w)] -> DRAM [h, j, w]
                        for blk in range(f):
                            t = tg * f + blk
                            nc.scalar.dma_start(
                                oview[c, t * TILE_P : (t + 1) * TILE_P],
                                O[:, blk * 4 * Ws : (blk + 1) * 4 * Ws]
                                .rearrange("p (j w) -> p j w", j=4),
                            )
    nc.compile()
    return nc


_CACHE = {}


def _get_program(w):
    key = w.tobytes()
    if key not in _CACHE:
        _CACHE[key] = _build(w)
    return _CACHE[key]


def _run(x, conv_weights, **spmd_kwargs):
    x = np.ascontiguousarray(np.asarray(x, dtype=np.float32))
    w = np.asarray(conv_weights, dtype=np.float32)
    assert x.shape == (B, C, H, W), x.shape
    nc = _get_program(w)
    in_maps = [
        {"x": x[k * BP : (k + 1) * BP].reshape(BP, C, Hs, BLK_F)}
        for k in range(N_CORES)
    ]
    res = run_bass_kernel_spmd(nc, in_maps, list(range(N_CORES)), **spmd_kwargs)
    out = np.concatenate([res.results[k]["out"] for k in range(N_CORES)], axis=0)
    return out.astype(np.float32, copy=False), res


def kernel(x, conv_weights):
    out, _ = _run(x, conv_weights)
    return out


def kernel_timed(x, conv_weights, **spmd_kwargs):
    """Run with NTFF profiling; returns (out, BassKernelResults)."""
    return _run(x, conv_weights, trace=True, **spmd_kwargs)
